# revision 21
# baseline (speedup 1.0000x reference)
"""Trainium2 Bass kernel for nn_MultiHeadAttention_21251498181338.

Music-Transformer-style MHA with relative position embeddings (Huang et al.
skew trick), B=2, L=2048, D=1024, H=16, causal mask.

Sharding: 8 cores = 2 batches x 4-head groups (tensor parallel per head).
Each core computes q/k/v projections for its 4 heads, causal attention with
relative position logits, and a partial output projection (Wo row-split).
Partials are summed on the host during unshard.

Device-side structure (per core), v2:
  - Projections produce qhT/khT/vhT in [head-depth on partitions] layout;
    vhT is then converted to [keys on partitions] layout (vh2) with a single
    xbar DMA-transpose per head pair, so the PV stationary operand needs no
    TensorE transposes.
  - Attention logits are computed directly TRANSPOSED: S^T = K Q^T with the
    key tile as the stationary operand, so the probabilities land in
    [keys on partitions, queries on free] layout and feed the PV matmul
    without any TensorE transpose of the probability band.
  - The relative-position term is computed as a raw-logit band X = Q E'^T in
    [queries, r] layout, padded with -30000, and moved into [keys, queries]
    layout by a single fused skew+transpose xbar DMA per 512-key chunk: the
    source access pattern uses partition step (row_len - 1), which applies
    the Huang et al. skew, and the xbar writes the result transposed. The
    -30000 padding lands exactly on the k > q positions, so the causal mask
    falls out of exp() for free.
  - P^T = exp((S^T + Srel^T) * scale) with a single exp per element
    (ScalarE), the add on VectorE.
  - PV and the softmax denominator accumulate into one PSUM tile pair with
    M=64 column-group packing: the two heads of a pair occupy complementary
    column groups so their matmuls run concurrently, and the denominator
    (all-ones stationary) lands on the same partitions as the output, making
    the normalization a single full-width vector multiply per 512 queries.
  - The attention output appears as outT [depth, queries], which is exactly
    the stationary-operand layout the output projection needs.
"""

import os
import sys

sys.path.insert(0, "/opt/trn_rl_repo")

import numpy as np
import ml_dtypes

import concourse.bass as bass
import concourse.mybir as mybir
import concourse.tile as tile
from concourse import bacc
from concourse.bass_utils import run_bass_kernel_spmd

BF16 = mybir.dt.bfloat16
F32 = mybir.dt.float32
NPBF16 = ml_dtypes.bfloat16

B, L, DM, H, D = 2, 2048, 1024, 16, 64
HG = 4            # heads per core (head group)
NCORES = 8
P = 128
KT = DM // P      # 8 contraction tiles for projections
NIT = L // P      # 16 query tiles
SCALE = 1.0 / np.sqrt(D)  # 0.125
NEG = -30000.0    # causal-mask pad value (exp(NEG*SCALE) == 0)

LAST_EXEC_NS = None

_PROG = None


def build_program():
    nc = bacc.Bacc(
        "TRN2",
        target_bir_lowering=False,
        debug=False,
        enable_asserts=False,
        num_devices=NCORES,
    )

    # ---- External I/O ----
    xq = nc.dram_tensor("xq", [DM, L], BF16, kind="ExternalInput")  # q[b].T
    xk = nc.dram_tensor("xk", [DM, L], BF16, kind="ExternalInput")
    xv = nc.dram_tensor("xv", [DM, L], BF16, kind="ExternalInput")
    wq = nc.dram_tensor("wq", [DM, 2 * P], BF16, kind="ExternalInput")  # group cols
    wk = nc.dram_tensor("wk", [DM, 2 * P], BF16, kind="ExternalInput")
    wv = nc.dram_tensor("wv", [DM, 2 * P], BF16, kind="ExternalInput")
    wo = nc.dram_tensor("wo", [2, P, DM], BF16, kind="ExternalInput")  # [hp, 2h*64, dm]
    eT = nc.dram_tensor("eT", [2, P, L], BF16, kind="ExternalInput")   # [hp, 2h*64, r]
    bqkv = nc.dram_tensor("bqkv", [P, 6], F32, kind="ExternalInput")
    bo_t = nc.dram_tensor("bo", [P, DM], F32, kind="ExternalInput")  # row-replicated
    out = nc.dram_tensor("out", [L, DM], F32, kind="ExternalOutput")

    with tile.TileContext(nc) as tc:
        with (
            tc.tile_pool(name="persist", bufs=1) as pp,
            tc.tile_pool(name="small", bufs=4) as sp,
        ):
            # ---- persistent SBUF tensors ----
            wo_sb = pp.tile([P, 2, DM], BF16)
            nc.sync.dma_start(wo_sb, wo.ap().rearrange("h p m -> p h m"))
            eT_sb = pp.tile([P, 2, L], BF16)
            nc.sync.dma_start(eT_sb, eT.ap().rearrange("h p r -> p h r"))
            bqkv_sb = pp.tile([P, 6], F32)
            nc.sync.dma_start(bqkv_sb, bqkv.ap())
            bo_sb = pp.tile([P, DM], F32)
            nc.sync.dma_start(bo_sb, bo_t.ap())

            qhT = pp.tile([P, 2, L], BF16)   # [64*hl+d, hp, i]
            khT = pp.tile([P, 2, L], BF16)
            vh2 = pp.tile([P, 2, NIT, P], BF16)  # [key in tile, hp, jt, 64*hl+d]
            outT = pp.tile([P, 2, L], BF16)  # [64*hl+d, hp, i]

            # all-ones stationary for the softmax-denominator matmul
            ones64 = pp.tile([P, 64], BF16)
            nc.gpsimd.memset(ones64, 1.0)

            # ---- Stage 1: projections ----
            with (
                tc.tile_pool(name="xin", bufs=2) as xp,
                tc.tile_pool(name="ps1", bufs=4, space="PSUM") as ps1,
            ):
                wq_sb = xp.tile([P, KT, 2 * P], BF16, tag="wq_sb")
                nc.sync.dma_start(wq_sb,
                                  wq.ap().rearrange("(t p) c -> p t c", p=P))
                wk_sb = xp.tile([P, KT, 2 * P], BF16, tag="wk_sb")
                nc.sync.dma_start(wk_sb,
                                  wk.ap().rearrange("(t p) c -> p t c", p=P))
                wv_sb = xp.tile([P, KT, 2 * P], BF16, tag="wv_sb")
                nc.sync.dma_start(wv_sb,
                                  wv.ap().rearrange("(t p) c -> p t c", p=P))
                vhT = xp.tile([P, 2, L], BF16, tag="vhT")
                for src, wsb, dst, bcol in (
                    (xq, wq_sb, qhT, 0), (xk, wk_sb, khT, 2), (xv, wv_sb, vhT, 4),
                ):
                    xt = xp.tile([P, KT, L], BF16, tag="xin")
                    nc.sync.dma_start(xt, src.ap().rearrange("(t p) i -> p t i", p=P))
                    for hp in range(2):
                        for ic in range(L // 512):
                            ps = ps1.tile([P, 512], F32, tag="ps1")
                            for kt in range(KT):
                                nc.tensor.matmul(
                                    ps,
                                    wsb[:, kt, hp * P:(hp + 1) * P],
                                    xt[:, kt, ic * 512:(ic + 1) * 512],
                                    start=(kt == 0),
                                    stop=(kt == KT - 1),
                                )
                            nc.scalar.activation(
                                dst[:, hp, ic * 512:(ic + 1) * 512], ps,
                                mybir.ActivationFunctionType.Identity,
                                bias=bqkv_sb[:, bcol + hp:bcol + hp + 1],
                            )
                # vhT [depth, keys] -> vh2 [keys, depth] via xbar transpose
                for hp in range(2):
                    nc.scalar.dma_start(
                        vh2[:, hp, :, :], vhT[:, hp, :], transpose=True
                    )

            # ---- Stage 2: attention ----
            # The PV stage runs one (hp, gq) step behind the band stage, so
            # the PE always has independent band matmuls to chew on while the
            # xbar transposes for the previous group complete (keeps HAM warm).
            with (
                tc.tile_pool(name="bandp", bufs=4) as bandp,
                tc.tile_pool(name="pep", bufs=3) as pep,
                tc.tile_pool(name="ptp", bufs=4) as ptp,
                tc.tile_pool(name="psx", bufs=3, space="PSUM") as psx,
                tc.tile_pool(name="psq", bufs=3, space="PSUM") as psqp,
                tc.tile_pool(name="pso", bufs=1, space="PSUM") as psop,
            ):
                def pv_gen(hp, gq, pT):
                    """Yield PV matmul steps for (hp, gq); final normalize.

                    Yielded as a generator so the caller can interleave the
                    PE instructions between the next group's band matmuls
                    (PE queue is strict FIFO — static order is everything).
                    """
                    NK = 4 * (gq + 1)
                    q0 = gq * 512
                    ps_o = psop.tile([P, 512], F32, tag="pso")
                    psd = psop.tile([P, 512], F32, tag="psd")
                    for jt in range(NK):
                        for hl in range(2):
                            pb_ = 64 * hl
                            nc.tensor.matmul(
                                ps_o[pb_:pb_ + 64, :],
                                vh2[:, hp, jt, pb_:pb_ + 64],
                                pT[hl][:, :, jt, :],
                                start=(jt == 0), stop=(jt == NK - 1),
                                skip_group_check=True,
                            )
                        for hl in range(2):
                            pb_ = 64 * hl
                            nc.tensor.matmul(
                                psd[pb_:pb_ + 64, :],
                                ones64,
                                pT[hl][:, :, jt, :],
                                start=(jt == 0), stop=(jt == NK - 1),
                                skip_group_check=True,
                            )
                        yield
                    # normalize both heads at once; denominators sit on the
                    # same partitions as the outputs
                    rec = sp.tile([P, 512], F32, tag="rec")
                    nc.vector.reciprocal_approx_fast(out=rec, in_=psd)
                    nc.vector.tensor_tensor(
                        outT[:, hp, q0:q0 + 512], ps_o, rec,
                        mybir.AluOpType.mult,
                    )
                    yield

                def bands_and_transposes(hp, gq, pv):
                    """Compute P^T tiles for (hp, gq), pulling steps off the
                    previous group's PV generator between chunks."""
                    NK = 4 * (gq + 1)
                    nj = NK * P
                    CW = nj + P
                    pT = [ptp.tile([P, 4, NK, P], BF16, tag="pT",
                                   name=f"pT{h_}") for h_ in range(2)]
                    pex = [pep.tile([P, 4 * nj], BF16, tag="pex",
                                    name=f"pex{h_}")
                           for h_ in range(2)]
                    for il in range(4):
                        it = 4 * gq + il
                        i0 = it * P
                        W = (it + 1) * P   # true causal band width
                        r_lo = L - P - i0
                        ncjt = (W + 511) // 512
                        q_stat = [qhT[64 * hl:64 * hl + 64, hp, i0:i0 + P]
                                  for hl in range(2)]
                        band = [bandp.tile([P, 2176], BF16, tag="band",
                                           name=f"band{h_}")
                                for h_ in range(2)]
                        # raw Srel band X in [q, r] layout (hl-interleaved
                        # so the K=64 matmuls pair across PE row groups)
                        for cs in range(ncjt):
                            n = min(512, W - cs * 512)
                            for hl in range(2):
                                ps = psx.tile([P, 512], F32, tag="psx")
                                nc.tensor.matmul(
                                    ps[:, :n],
                                    q_stat[hl],
                                    eT_sb[64 * hl:64 * hl + 64, hp,
                                          r_lo + cs * 512:r_lo + cs * 512 + n],
                                    start=True, stop=True,
                                )
                                nc.any.tensor_copy(
                                    band[hl][:, cs * 512:cs * 512 + n],
                                    ps[:, :n],
                                )
                            next(pv, None)
                        for hl in range(2):
                            nc.gpsimd.memset(band[hl][:, W:CW], NEG)
                            # skew: [q, r] -> [q, k], -30000 where k > q
                            # (issued via SWDGE so the HWDGE queues keep
                            # serving the xbar transposes and loads)
                            row_len = band[hl].ap[0][0]
                            diag = bass.AP(
                                band[hl].tensor, band[hl].offset + 127,
                                [[row_len - 1, P], [1, nj]],
                            )
                            nc.gpsimd.dma_start(
                                pex[hl][:, il * nj:(il + 1) * nj], diag)
                        # logits += QK^T (in place, true causal width)
                        for jc in range(ncjt):
                            n = min(512, W - jc * 512)
                            for hl in range(2):
                                ps = psqp.tile([P, 512], F32, tag="psq")
                                nc.tensor.matmul(
                                    ps[:, :n],
                                    q_stat[hl],
                                    khT[64 * hl:64 * hl + 64, hp,
                                        jc * 512:jc * 512 + n],
                                    start=True, stop=True,
                                )
                                nc.vector.tensor_tensor(
                                    pex[hl][:, il * nj + jc * 512:
                                            il * nj + jc * 512 + n],
                                    ps[:, :n],
                                    pex[hl][:, il * nj + jc * 512:
                                            il * nj + jc * 512 + n],
                                    mybir.AluOpType.add,
                                )
                            next(pv, None)
                    for hl in range(2):
                        # one exp + one xbar transpose per (hp, gq, hl):
                        # the concatenated band [q, il*nj + k] transposes
                        # tile-by-tile straight into pT's [il, jt] layout
                        nc.scalar.activation(
                            pex[hl], pex[hl],
                            mybir.ActivationFunctionType.Exp, scale=SCALE,
                        )
                        nc.sync.dma_start(
                            pT[hl][:, :, :, :], pex[hl][:, :],
                            transpose=True,
                        )
                    return pT

                seq = [(hp, gq) for hp in range(2) for gq in range(4)]
                pv = iter(())
                for hp, gq in seq:
                    pT = bands_and_transposes(hp, gq, pv)
                    for _ in pv:  # flush any remaining PV steps
                        pass
                    pv = pv_gen(hp, gq, pT)
                for _ in pv:
                    pass

            # ---- Stage 3: output projection (partial: this head group) ----
            with tc.tile_pool(name="ps3", bufs=2, space="PSUM") as ps3:
                for it in range(NIT):
                    pss3 = [ps3.tile([P, 512], F32, tag=f"ps3{mc}",
                                     name=f"ps3{mc}")
                            for mc in range(2)]
                    for hp in range(2):
                        for mc in range(2):
                            nc.tensor.matmul(
                                pss3[mc],
                                outT[:, hp, it * P:(it + 1) * P],
                                wo_sb[:, hp, mc * 512:(mc + 1) * 512],
                                start=(hp == 0),
                                stop=(hp == 1),
                            )
                    for mc in range(2):
                        osb = sp.tile([P, 512], F32, tag="osb")
                        nc.vector.tensor_tensor(
                            osb, pss3[mc], bo_sb[:, mc * 512:(mc + 1) * 512],
                            mybir.AluOpType.add,
                        )
                        nc.sync.dma_start(
                            out.ap()[it * P:(it + 1) * P,
                                     mc * 512:(mc + 1) * 512], osb
                        )
    nc.compile()
    return nc


def _prep_inputs(q, k, v, Wq, bq, Wk, bk, Wv, bv, Wo, bo, E):
    """Build the 8 per-core input maps (host-side shard + cast)."""
    in_maps = []
    for core in range(NCORES):
        b, g = core // HG, core % HG
        cols = slice(g * HG * D, (g + 1) * HG * D)  # this group's 256 cols
        # eT/wo packing: [hp, 64*hl + d, .]
        eTg = np.empty((2, P, L), NPBF16)
        wog = np.empty((2, P, DM), NPBF16)
        for hp in range(2):
            for hl in range(2):
                h = g * HG + 2 * hp + hl
                eTg[hp, 64 * hl:64 * hl + 64, :] = E[:, h * D:(h + 1) * D].T.astype(NPBF16)
                wog[hp, 64 * hl:64 * hl + 64, :] = Wo[h * D:(h + 1) * D, :].astype(NPBF16)
        bqkv_a = np.empty((P, 6), np.float32)
        for hp in range(2):
            sl = slice(g * HG * D + hp * P, g * HG * D + (hp + 1) * P)
            bqkv_a[:, hp] = bq[sl]
            bqkv_a[:, 2 + hp] = bk[sl]
            bqkv_a[:, 4 + hp] = bv[sl]
        bo_full = bo if g == 0 else np.zeros_like(bo)
        in_maps.append({
            "xq": np.ascontiguousarray(q[b].T).astype(NPBF16),
            "xk": np.ascontiguousarray(k[b].T).astype(NPBF16),
            "xv": np.ascontiguousarray(v[b].T).astype(NPBF16),
            "wq": np.ascontiguousarray(Wq[:, cols]).astype(NPBF16),
            "wk": np.ascontiguousarray(Wk[:, cols]).astype(NPBF16),
            "wv": np.ascontiguousarray(Wv[:, cols]).astype(NPBF16),
            "wo": wog,
            "eT": eTg,
            "bqkv": bqkv_a,
            "bo": np.ascontiguousarray(
                np.broadcast_to(bo_full[None, :], (P, DM))).astype(np.float32),
        })
    return in_maps


def _reference_numpy(q, k, v, mask, Wq, bq, Wk, bk, Wv, bv, Wo, bo, E):
    """Exact fallback for non-causal masks (never hit in practice)."""
    def split_heads(x):
        return np.moveaxis(x.reshape(*x.shape[:-1], H, D), -2, -3)
    qh = split_heads(q @ Wq + bq)
    kh = split_heads(k @ Wk + bk)
    vv = split_heads(v @ Wv + bv)
    eh = split_heads(E)
    QKt = np.einsum("bhqd,bhkd->bhqk", qh, kh)
    X = np.einsum("bhqd,hkd->bhqk", qh, eh)
    pad = np.pad(X, [(0, 0)] * 3 + [(1, 0)])
    s = pad.reshape(B, H, -1)[:, :, L:].reshape(B, H, L, L)
    logits = (QKt + s) / np.sqrt(D) + mask * -1e9
    m = logits.max(-1, keepdims=True)
    p = np.exp(logits - m)
    p /= p.sum(-1, keepdims=True)
    o = np.einsum("bhqk,bhkd->bhqd", p, vv)
    o = np.moveaxis(o, -3, -2).reshape(B, L, DM)
    return (o @ Wo + bo).astype(np.float32)


def benchmark(inputs, iters=20):
    """Amortized wall-clock of the sharded NEFF execution (device-resident
    inputs, back-to-back async dispatch). Returns est. ns per execution."""
    global _PROG
    import time as _time
    import jax
    from jax.sharding import Mesh, PartitionSpec
    from jax.experimental.shard_map import shard_map
    import concourse.bass2jax as b2j
    import concourse.mybir as mb

    if _PROG is None:
        _PROG = build_program()
    nc = _PROG
    args = {n: np.asarray(inputs[n], np.float32)
            for n in ("q", "k", "v", "Wq", "bq", "Wk", "bk", "Wv", "bv",
                      "Wo", "bo", "E")}
    in_maps = _prep_inputs(**args)
    b2j.install_neuronx_cc_hook()

    partition_name = (nc.partition_id_tensor.name
                      if nc.partition_id_tensor else None)
    in_names, out_names, out_avals, zero_outs = [], [], [], []
    for alloc in nc.m.functions[0].allocations:
        if not isinstance(alloc, mb.MemoryLocationSet):
            continue
        name = alloc.memorylocations[0].name
        if alloc.kind == "ExternalInput":
            if name != partition_name:
                in_names.append(name)
        elif alloc.kind == "ExternalOutput":
            out_names.append(name)
            shape = tuple(alloc.tensor_shape)
            dtype = mb.dt.np(alloc.dtype)
            out_avals.append(jax.core.ShapedArray(shape, dtype))
            zero_outs.append(np.zeros(shape, dtype))
    n_params = len(in_names)
    n_outs = len(out_avals)
    all_names = in_names + out_names
    if partition_name is not None:
        all_names = all_names + [partition_name]

    def _body(*fargs):
        operands = list(fargs)
        if partition_name is not None:
            operands.append(b2j.partition_id_tensor())
        outs = b2j._bass_exec_p.bind(
            *operands, out_avals=tuple(out_avals), in_names=tuple(all_names),
            out_names=tuple(out_names), lowering_input_output_aliases=(),
            sim_require_finite=True, sim_require_nnan=True, nc=nc)
        return tuple(outs)

    devices = jax.devices()[:NCORES]
    mesh = Mesh(np.asarray(devices), ("core",))
    in_specs = (PartitionSpec("core"),) * (n_params + n_outs)
    out_specs = (PartitionSpec("core"),) * n_outs
    sharded = jax.jit(
        shard_map(_body, mesh=mesh, in_specs=in_specs, out_specs=out_specs,
                  check_rep=False),
        keep_unused=True)

    concat_in = [np.concatenate([np.asarray(in_maps[c][n])
                                 for c in range(NCORES)], axis=0)
                 for n in in_names]
    dev_in = [jax.device_put(a) for a in concat_in]
    concat_zero = [np.concatenate([z] * NCORES, axis=0) for z in zero_outs]

    dev_zero = [jax.device_put(z) for z in concat_zero]
    # warmup (compiles / caches)
    outs = sharded(*dev_in, *dev_zero)
    jax.block_until_ready(outs)

    t0 = _time.perf_counter()
    results = []
    for _ in range(iters):
        results.append(sharded(*dev_in, *dev_zero))
    jax.block_until_ready(results)
    t1 = _time.perf_counter()
    return (t1 - t0) / iters * 1e9


def kernel(**inputs):
    global _PROG, LAST_EXEC_NS
    args = {n: np.asarray(inputs[n], np.float32)
            for n in ("q", "k", "v", "Wq", "bq", "Wk", "bk", "Wv", "bv",
                      "Wo", "bo", "E")}
    mask = np.asarray(inputs["mask"], np.float32)

    causal = np.array_equal(mask, np.triu(np.ones((L, L), np.float32), k=1))
    if not causal:
        return _reference_numpy(mask=mask, **args)

    if _PROG is None:
        _PROG = build_program()
    in_maps = _prep_inputs(**args)
    trace = os.environ.get("KERNEL_TRACE", "0") == "1"
    try:
        res = run_bass_kernel_spmd(_PROG, in_maps, core_ids=list(range(NCORES)),
                                   trace=trace)
    except ModuleNotFoundError:
        # axon NTFF profiling hook unavailable in this container
        res = run_bass_kernel_spmd(_PROG, in_maps, core_ids=list(range(NCORES)),
                                   trace=False)
    LAST_EXEC_NS = res.exec_time_ns
    globals()["LAST_RESULTS"] = res

    full = np.zeros((B, L, DM), np.float32)
    for core in range(NCORES):
        full[core // HG] += res.results[core]["out"]
    return full
